# revision 19
# baseline (speedup 1.0000x reference)
"""GraphSAGE 2-layer encoder on 8 Trainium2 NeuronCores (Bass/Tile).

Strategy (graph/data parallel, dst-sharded):
  - Nodes are range-sharded across 8 cores (6250 each, padded to 6400).
  - Mean aggregation runs on the TensorEngine via one-hot selection
    matrices: for each chunk of 128 edges, sel[e, n] = (dst_local[e]==n).
    All sel chunks of a (supertile, 128-subtile) are built by ONE wide
    DVE tensor_tensor is_equal with broadcast access patterns
    (in0 = iota row tile broadcast across chunks, in1 = per-chunk dst
    columns broadcast across the 128 one-hot lanes) — no per-chunk
    tensor_scalar, no fp32 scalar tables.
  - The 1/deg mean normalization is NOT baked into sel. Layer 1 folds it
    into the PSUM->SBUF move (one DVE tensor_tensor mult against a
    replicated-invdeg table). Layer 2 accumulates agg row-major
    (lhsT=sel) and folds normalization + accumulation into the output
    PSUM group with a single matmul against a diagonal-invdeg tile.
  - Per-edge source features are fetched with gpsimd dma_gather (int16
    indices, so gather tables are split in two <32768-row halves).
  - Layer 2 uses the projection identity agg2 @ W2b = D^-1 A2 (h @ W2b):
    each core projects its own h rows to p = h @ W2b (128-dim) on the
    PE (row-major, directly from the resident transposed h), AllGathers
    p (half the bytes of h), and gathers 256-byte p rows per edge.
    out^T = relu(W2a^T h^T + D^-1 aggp^T + b2) is produced transposed
    [128, NPAD]; the host transposes back.
  - The p AllGather is split in two chunks (separate Shared tensors,
    which double as the int16 lo/hi gather-table halves); chunk 0 is
    issued as soon as supertile 12 of layer 1 completes so it overlaps
    the tail of layer 1.

The Bass program is identical for all cores and independent of the edge
data (given fixed chunk capacities) — per-core behavior comes only from
the input tables.
"""

import numpy as np
import ml_dtypes

import concourse.bass as bass
import concourse.mybir as mybir
import concourse.tile as tile
from concourse import bacc
from concourse.bass import AP
from concourse.bass_utils import run_bass_kernel_spmd
from concourse.masks import make_identity

BF16 = ml_dtypes.bfloat16

# problem constants (hardcoded per contract)
N = 50000
E = 800000
IN_DIM = 128
HID = 256
OUT_DIM = 128

NCORES = 8
NPC = N // NCORES          # 6250 nodes per core
ST = 256                   # supertile (dst nodes per outer loop iteration)
NST = 25                   # supertiles per core (6400 padded rows)
NPAD = NST * ST            # 6400
HALF_X = N // 2            # 25000: layer-1 gather-table half size
BLK0 = NPC // 2            # 3125: allgather chunk-0 rows per core
BLK1 = NPC - BLK0          # 3125: chunk-1 rows (exposed tail collective)
P = 128

CSUB_DEFAULT = 9           # chunks of 128 edges per (128-subtile, half)
GSPLIT = 6                 # chunks per dma_gather instruction
AG0_ST = 12                # first supertile covering all of chunk-0 rows
LAG = 3                    # layer-2 half-1 gather delay (supertiles)

_PROGRAM_CACHE: dict = {}


def _bcast_free(ap, reps, axis_pos):
    """Insert a [0, reps] broadcast free dim at axis_pos into an AP."""
    new = AP(ap.tensor, ap.offset, [list(p) for p in ap.ap])
    new.ap.insert(axis_pos, [0, reps])
    return new


# ----------------------------------------------------------------------------
# host-side preprocessing
# ----------------------------------------------------------------------------

def _pack_layer(src, dst, half_of, rel_of, csub_default):
    """Build per-core gather/selection tables for one layer.

    Edges are grouped by (core, 256-supertile, table-half, 128-subtile),
    each group padded to CSUB chunks of 128 edges (pad edges gather row 0,
    dst slot 0 of a zero-weight... pad slots point at one-hot column of
    dst 0 but their gathered row is multiplied into column 0 only via
    sel; normalization tables make pad contributions vanish because pad
    slots use dst value -1 (no one-hot match)).

    Column orders differ per consumer:
      gather idx cols:  (st, half, sub, c)  [h-pure runs for dma_gather]
      sel dst cols:     (st, sub, half, c)  [contiguous run per wide TT]

    Returns (CSUB, idx_tabs, dst_tabs); per core:
      idx_tab [128, NST*2*(2*CSUB)*8] int16 (16-partition wrap, tiled x8)
      dst_tab [128, NST*4*CSUB] bf16 (dst one-hot targets, -1 for pads)
    """
    core = dst // NPC
    loc = dst - core * NPC                   # 0..6249
    st = loc >> 8                            # supertile 0..24
    sub = (loc >> 7) & 1                     # 128-subtile within supertile
    dst_in = (loc & 127).astype(np.float32)
    half = half_of[src].astype(np.int64)
    rel = rel_of[src].astype(np.int16)

    group = ((core * NST + st) * 2 + half) * 2 + sub
    ngroups = NCORES * NST * 4
    counts = np.bincount(group, minlength=ngroups)
    CSUB = max(csub_default, int(-(-counts.max() // P)))
    cap = CSUB * P

    order = np.argsort(group, kind="stable")
    g_sorted = group[order]
    starts = np.concatenate([[0], np.cumsum(counts)])
    slot = np.arange(E) - starts[g_sorted]

    idx_pad = np.zeros((ngroups, cap), dtype=np.int16)
    dst_pad = np.full((ngroups, cap), -1.0, dtype=np.float32)
    idx_pad[g_sorted, slot] = rel[order]
    dst_pad[g_sorted, slot] = dst_in[order]

    # group index decomposition: g = ((cs * NST + st) * 2 + h) * 2 + sub
    gs = np.arange(ngroups)
    g_core, g_rem = gs // (NST * 4), gs % (NST * 4)
    g_st, g_hs = g_rem // 4, g_rem % 4
    g_h, g_sub = g_hs // 2, g_hs % 2

    # idx wrap: linear i -> (partition i%16, col i//16); [16, CSUB*8] per group
    idx_w = idx_pad.reshape(ngroups, CSUB * 8, 16).transpose(0, 2, 1)
    # dst chunk cols: linear i -> (partition i%128, chunk i//128)
    dst_w = dst_pad.reshape(ngroups, CSUB, P).transpose(0, 2, 1)   # [g,128,CSUB]

    # gather order (st, h, sub, c) / sel order (st, sub, h, c)
    gath_order = np.lexsort((g_sub, g_h, g_st, g_core))
    sel_order = np.lexsort((g_h, g_sub, g_st, g_core))

    idx_tabs, dst_tabs = [], []
    npg = NST * 4
    for c in range(NCORES):
        go = gath_order[c * npg:(c + 1) * npg]
        so = sel_order[c * npg:(c + 1) * npg]
        it = idx_w[go].transpose(1, 0, 2).reshape(16, -1)
        idx_tabs.append(np.ascontiguousarray(np.tile(it, (8, 1))))
        dst_tabs.append(np.ascontiguousarray(
            dst_w[so].transpose(1, 0, 2).reshape(P, -1)).astype(BF16))
    return CSUB, idx_tabs, dst_tabs


def _invdeg_tables(dst_edges):
    """Per-core [128, NPAD] bf16: replicated-row and diagonal invdeg."""
    deg = np.bincount(dst_edges, minlength=N)
    invdeg = (1.0 / np.maximum(deg, 1.0)).astype(np.float32)
    rep_tabs, diag_tabs = [], []
    for c in range(NCORES):
        v = np.zeros(NPAD, np.float32)
        v[:NPC] = invdeg[c * NPC:(c + 1) * NPC]
        rep = np.broadcast_to(v, (P, NPAD))
        dia = np.zeros((P, NPAD), np.float32)
        blocks = v.reshape(NPAD // P, P)
        for b in range(NPAD // P):
            dia[:, b * P:(b + 1) * P] = np.diag(blocks[b])
        rep_tabs.append(np.ascontiguousarray(rep).astype(BF16))
        diag_tabs.append(np.ascontiguousarray(dia).astype(BF16))
    return rep_tabs, diag_tabs


def _preprocess(x, W1, b1, W2, b2, es0, ed0, es1, ed1):
    x = np.asarray(x, dtype=np.float32)
    es0 = np.asarray(es0, dtype=np.int64)
    ed0 = np.asarray(ed0, dtype=np.int64)
    es1 = np.asarray(es1, dtype=np.int64)
    ed1 = np.asarray(ed1, dtype=np.int64)

    # layer 1: table is x itself, node n at row n; halves split at 25000
    ident_rows = np.arange(N, dtype=np.int64)
    half1 = (ident_rows >= HALF_X).astype(np.int64)
    rel1 = ident_rows - half1 * HALF_X
    CS1, idx1, dst1 = _pack_layer(es0, ed0, half1, rel1, CSUB_DEFAULT)
    rep1, _ = _invdeg_tables(ed0)

    # layer 2: two p_full chunks: node n = c*NPC + j lives in chunk 0 at
    # row c*BLK0 + j (j < BLK0), else chunk 1 at c*BLK1 + (j - BLK0)
    c_of = ident_rows // NPC
    j_of = ident_rows % NPC
    half2 = (j_of >= BLK0).astype(np.int64)
    rel2 = np.where(half2 == 0, c_of * BLK0 + j_of,
                    c_of * BLK1 + (j_of - BLK0))
    CS2, idx2, dst2 = _pack_layer(es1, ed1, half2, rel2, CSUB_DEFAULT)
    _, diag2 = _invdeg_tables(ed1)

    x_bf = x.astype(BF16)
    xownt = []
    for c in range(NCORES):
        xo = np.zeros((NPAD, IN_DIM), dtype=np.float32)
        xo[:NPC] = x[c * NPC:(c + 1) * NPC]
        xownt.append(np.ascontiguousarray(xo.T).astype(BF16))

    W1_bf = np.asarray(W1, np.float32).astype(BF16)           # [256, 256]
    W2_bf = np.asarray(W2, np.float32).astype(BF16)           # [512, 128]
    b1_2 = np.asarray(b1, np.float32).reshape(2, P).T.copy()  # [128, 2]
    b2_c = np.asarray(b2, np.float32).reshape(P, 1).copy()    # [128, 1]

    in_maps = []
    for c in range(NCORES):
        in_maps.append({
            "xtab": x_bf,
            "xownt": xownt[c],
            "w1": W1_bf,
            "w2": W2_bf,
            "b1": b1_2,
            "b2": b2_c,
            "idx1": idx1[c], "dst1": dst1[c], "rep1": rep1[c],
            "idx2": idx2[c], "dst2": dst2[c], "diag2": diag2[c],
        })
    return CS1, CS2, in_maps


# ----------------------------------------------------------------------------
# device program
# ----------------------------------------------------------------------------

def build_program(CS1, CS2, ablate=()):
    key = (CS1, CS2, tuple(sorted(ablate)))
    if key in _PROGRAM_CACHE:
        return _PROGRAM_CACHE[key]

    C1, C2 = 2 * CS1, 2 * CS2          # chunks per (supertile, half)
    S1, S2 = 2 * C1, 2 * C2            # chunk slots per supertile
    dt = mybir.dt
    AF = mybir.ActivationFunctionType
    nc = bacc.Bacc("TRN2", target_bir_lowering=False, debug=False,
                   num_devices=NCORES, num_swdge_queues=4,
                   dynamic_dma_scratch_size=32768)

    t_xtab = nc.dram_tensor("xtab", [N, IN_DIM], dt.bfloat16, kind="ExternalInput")
    t_xownt = nc.dram_tensor("xownt", [IN_DIM, NPAD], dt.bfloat16, kind="ExternalInput")
    t_w1 = nc.dram_tensor("w1", [HID, HID], dt.bfloat16, kind="ExternalInput")
    t_w2 = nc.dram_tensor("w2", [2 * HID, OUT_DIM], dt.bfloat16, kind="ExternalInput")
    t_b1 = nc.dram_tensor("b1", [P, 2], dt.float32, kind="ExternalInput")
    t_b2 = nc.dram_tensor("b2", [P, 1], dt.float32, kind="ExternalInput")
    t_idx1 = nc.dram_tensor("idx1", [P, NST * 2 * C1 * 8], dt.int16, kind="ExternalInput")
    t_dst1 = nc.dram_tensor("dst1", [P, NST * S1], dt.bfloat16, kind="ExternalInput")
    t_rep1 = nc.dram_tensor("rep1", [P, NPAD], dt.bfloat16, kind="ExternalInput")
    t_idx2 = nc.dram_tensor("idx2", [P, NST * 2 * C2 * 8], dt.int16, kind="ExternalInput")
    t_dst2 = nc.dram_tensor("dst2", [P, NST * S2], dt.bfloat16, kind="ExternalInput")
    t_diag2 = nc.dram_tensor("diag2", [P, NPAD], dt.bfloat16, kind="ExternalInput")
    t_out = nc.dram_tensor("out", [OUT_DIM, NPAD], dt.bfloat16, kind="ExternalOutput")

    qctr = [0]
    with tile.TileContext(nc) as tc:
        with tc.tile_pool(name="const", bufs=1) as cp, \
             tc.tile_pool(name="dram", bufs=1, space="DRAM") as dp:

            # ---- constants / persistent SBUF ----
            ident_bf = cp.tile([P, P], dt.bfloat16, name="ident_bf")
            make_identity(nc, ident_bf)
            iota_i = cp.tile([P, P], dt.int32, name="iota_i")
            nc.gpsimd.iota(iota_i, pattern=[[1, P]], base=0, channel_multiplier=0)
            iota_bf = cp.tile([P, P], dt.bfloat16, name="iota_bf")
            nc.vector.tensor_copy(iota_bf[:], iota_i[:])

            # layer-1 gather indices load first (and in two pieces) so the
            # first gathers issue within a few microseconds of program start
            idx1_sb = cp.tile([P, NST * 2 * C1 * 8], dt.int16, name="idx1_sb")
            c_split = 4 * C1 * 8
            nc.sync.dma_start(idx1_sb[:, 0:c_split], t_idx1.ap()[:, 0:c_split])
            nc.sync.dma_start(idx1_sb[:, c_split:], t_idx1.ap()[:, c_split:])

            # w2 rows: a=0,1 -> W2a halves; a=2,3 -> W2b halves
            w1_sb = cp.tile([P, 2, HID], dt.bfloat16, name="w1_sb")
            nc.sync.dma_start(w1_sb[:], t_w1.ap().rearrange("(a p) h -> p a h", p=P))
            w2_sb = cp.tile([P, 4, OUT_DIM], dt.bfloat16, name="w2_sb")
            nc.sync.dma_start(w2_sb[:], t_w2.ap().rearrange("(a p) h -> p a h", p=P))
            b1_sb = cp.tile([P, 2], dt.float32, name="b1_sb")
            nc.sync.dma_start(b1_sb[:], t_b1.ap()[:])
            b2_sb = cp.tile([P, 1], dt.float32, name="b2_sb")
            nc.sync.dma_start(b2_sb[:], t_b2.ap()[:])

            xt_all = cp.tile([P, NPAD], dt.bfloat16, name="xt_all")
            nc.sync.dma_start(xt_all[:], t_xownt.ap()[:])
            dst1_sb = cp.tile([P, NST * S1], dt.bfloat16, name="dst1_sb")
            nc.sync.dma_start(dst1_sb[:], t_dst1.ap()[:])
            rep1_sb = cp.tile([P, NPAD], dt.bfloat16, name="rep1_sb")
            nc.sync.dma_start(rep1_sb[:], t_rep1.ap()[:])
            idx2_sb = cp.tile([P, NST * 2 * C2 * 8], dt.int16, name="idx2_sb")
            nc.sync.dma_start(idx2_sb[:], t_idx2.ap()[:])
            dst2_sb = cp.tile([P, NST * S2], dt.bfloat16, name="dst2_sb")
            nc.sync.dma_start(dst2_sb[:], t_dst2.ap()[:])
            diag2_sb = cp.tile([P, NPAD], dt.bfloat16, name="diag2_sb")
            nc.sync.dma_start(diag2_sb[:], t_diag2.ap()[:])

            # persistent transposed h (self-features for layer 2)
            hta = cp.tile([P, NPAD], dt.bfloat16, name="hta")
            htb = cp.tile([P, NPAD], dt.bfloat16, name="htb")

            # allgather chunk shards + gathered tables
            blks = (BLK0, BLK1)
            pshs = [
                dp.tile([blks[k], OUT_DIM], dt.bfloat16, name=f"psh{k}")
                for k in range(2)
            ]
            pfulls = [
                dp.tile([NCORES * blks[k], OUT_DIM], dt.bfloat16,
                        name=f"pfull{k}", addr_space="Shared")
                for k in range(2)
            ]

            def do_allgather(k):
                nc.gpsimd.collective_compute(
                    "AllGather",
                    mybir.AluOpType.bypass,
                    replica_groups=[list(range(NCORES))],
                    ins=[pshs[k][:].opt()],
                    outs=[pfulls[k][:].opt()],
                )

            def build_sel_wide(sp, dst_sb, st, sub, CS, tag, half=None, bufs=3):
                """One wide TT is_equal building one-hot sel chunks.

                half=None: all 2*CS chunks of (st, sub), dst cols ordered
                (st, sub, h, c). half=h: the CS chunks of (st, sub=0/1, h)
                for BOTH subs -> [128, 2*CS, 128] with chunk dim (sub, c),
                using a 4-dim strided in1 (cols (st, sub, h, c))."""
                sel = sp.tile([P, 2 * CS, P], dt.bfloat16, name=tag, tag=tag, bufs=bufs)
                if half is None:
                    base = (st * 2 + sub) * (2 * CS)
                    in0 = _bcast_free(iota_bf[:], 2 * CS, 1)
                    in1 = _bcast_free(dst_sb[:, base:base + 2 * CS], P, 2)
                else:
                    base = (st * 4 + half) * CS
                    in0 = _bcast_free(_bcast_free(iota_bf[:], CS, 1), 2, 1)
                    cols = dst_sb[:, base:base + CS]
                    # dims: [part, (sub: stride 2*CS cols), (c: 1), (bcast 128)]
                    cstride = cols.ap[-1][0]
                    in1 = AP(cols.tensor, cols.offset,
                             [list(cols.ap[0]), [2 * CS * cstride, 2],
                              [cstride, CS], [0, P]])
                nc.vector.tensor_tensor(out=sel[:], in0=in0, in1=in1,
                                        op=mybir.AluOpType.is_equal)
                return sel

            # ---- layer 1 ----
            with tc.tile_pool(name="l1sb", bufs=2) as sp, \
                 tc.tile_pool(name="l1ps", bufs=2, space="PSUM") as pp:
                for st in range(NST):
                    r0 = st * ST

                    # gathers: chunk dim of gat is (sub, c) within half h
                    gats = []
                    for h in range(2):
                        g = st * 2 + h
                        gat = sp.tile([P, C1, IN_DIM], dt.bfloat16,
                                      name=f"gat{h}", tag=f"gat{h}", bufs=4)
                        if "gather" in ablate:
                            nc.vector.memset(gat[:], 0.0)
                        for j in range(0, C1, GSPLIT):
                            if "gather" in ablate:
                                continue
                            w = min(GSPLIT, C1 - j)
                            nc.gpsimd.dma_gather(
                                out_ap=gat[:, j:j + w, :],
                                in_ap=t_xtab.ap()[h * HALF_X:(h + 1) * HALF_X, :],
                                idxs_ap=idx1_sb[:, (g * C1 + j) * 8:(g * C1 + j + w) * 8],
                                num_idxs=w * P,
                                num_idxs_reg=w * P,
                                elem_size=IN_DIM,
                                queue_num=qctr[0] % 4,
                            )
                            qctr[0] += 1
                        gats.append(gat)

                    # one [128,128] psum tile per 128-subtile, A then B
                    aggT = sp.tile([P, ST], dt.bfloat16, name="aggT", tag="aggT")
                    for sub in range(2):
                        sel = build_sel_wide(sp, dst1_sb, st, sub, CS1, "sel")
                        aggT_ps = pp.tile([P, P], dt.float32, name="aggT_ps", tag="aggT_ps", bufs=3)
                        n_mm = 2 * CS1
                        k = 0
                        for h in range(2):
                            for c in range(CS1):
                                nc.tensor.matmul(
                                    aggT_ps[:],
                                    lhsT=gats[h][:, sub * CS1 + c, :],
                                    rhs=sel[:, h * CS1 + c, :],
                                    start=(k == 0), stop=(k == n_mm - 1))
                                k += 1
                        # fused PSUM->SBUF copy + per-node invdeg scale
                        nc.vector.tensor_tensor(
                            out=aggT[:, sub * P:(sub + 1) * P], in0=aggT_ps[:],
                            in1=rep1_sb[:, r0 + sub * P:r0 + (sub + 1) * P],
                            op=mybir.AluOpType.mult)

                    # hT = relu(W1^T @ [x; agg] + b1), two hid halves
                    for hh, hstore in ((0, hta), (1, htb)):
                        hT_ps = pp.tile([P, ST], dt.float32, name="hT_ps", tag="hT_ps")
                        nc.tensor.matmul(hT_ps[:], lhsT=w1_sb[:, 0, hh * P:(hh + 1) * P],
                                         rhs=xt_all[:, r0:r0 + ST], start=True, stop=False)
                        nc.tensor.matmul(hT_ps[:], lhsT=w1_sb[:, 1, hh * P:(hh + 1) * P],
                                         rhs=aggT[:], start=False, stop=True)
                        nc.scalar.activation(hstore[:, r0:r0 + ST], hT_ps[:],
                                             AF.Relu, bias=b1_sb[:, hh:hh + 1])

                    # p rows = h @ W2b (row-major via lhsT = resident hT)
                    for nh in range(2):
                        rr = r0 + nh * P
                        if rr >= NPC:
                            continue
                        p_ps = pp.tile([P, OUT_DIM], dt.float32, name="p_ps", tag="p_ps", bufs=3)
                        nc.tensor.matmul(p_ps[:], lhsT=hta[:, rr:rr + P],
                                         rhs=w2_sb[:, 2, :], start=True, stop=False)
                        nc.tensor.matmul(p_ps[:], lhsT=htb[:, rr:rr + P],
                                         rhs=w2_sb[:, 3, :], start=False, stop=True)
                        p_sb = sp.tile([P, OUT_DIM], dt.bfloat16, name="p_sb", tag="p_sb", bufs=3)
                        nc.scalar.activation(p_sb[:], p_ps[:], AF.Copy)
                        # split the store at the allgather chunk boundary
                        for seg_lo, seg_hi, t_dst in ((0, BLK0, pshs[0]),
                                                      (BLK0, NPC, pshs[1])):
                            lo = max(rr, seg_lo)
                            hi = min(rr + P, seg_hi)
                            if hi > lo:
                                nc.sync.dma_start(
                                    t_dst[lo - seg_lo:hi - seg_lo, :],
                                    p_sb[lo - rr:hi - rr, :])

                    if st == AG0_ST:
                        do_allgather(0)

            do_allgather(1)

            # ---- layer 2 ----
            # aggp accumulates row-major [node, out] (lhsT=sel chunks); the
            # output PSUM group stacks the dense self terms then one matmul
            # against the diagonal-invdeg tile which both normalizes and
            # accumulates the aggregation term.
            with tc.tile_pool(name="l2sb", bufs=2) as sp, \
                 tc.tile_pool(name="l2ps", bufs=2, space="PSUM") as pp:
                def issue_gathers(st, h):
                    g = st * 2 + h
                    gat = sp.tile([P, C2, OUT_DIM], dt.bfloat16,
                                  name=f"g2_{h}", tag=f"g2_{h}",
                                  bufs=4 if h == 0 else 3)
                    if "gather" in ablate:
                        nc.vector.memset(gat[:], 0.0)
                        return gat
                    for j in range(0, C2, GSPLIT):
                        w = min(GSPLIT, C2 - j)
                        nc.gpsimd.dma_gather(
                            out_ap=gat[:, j:j + w, :],
                            in_ap=pfulls[h][:],
                            idxs_ap=idx2_sb[:, (g * C2 + j) * 8:(g * C2 + j + w) * 8],
                            num_idxs=w * P,
                            num_idxs_reg=w * P,
                            elem_size=OUT_DIM,
                            queue_num=qctr[0] % 4,
                        )
                        qctr[0] += 1
                    return gat

                # Half-0 gathers run LAG supertiles ahead so the Pool engine
                # keeps gathering from pfull0 while the chunk-1 allgather is
                # still in flight (in-order queue would otherwise block on the
                # first pfull1 gather). The aggregation PSUM is split per half
                # so half-0 matmuls consume (and release) their gather tiles
                # without waiting on half-1.
                def agg_half(st, h, gat):
                    sel = build_sel_wide(sp, dst2_sb, st, 2, CS2, f"sel2_{h}",
                                         half=h, bufs=2)
                    aggs = []
                    for sub in range(2):
                        aggp_ps = pp.tile([P, OUT_DIM], dt.float32,
                                          name=f"aggp{h}", tag=f"aggp{h}", bufs=2)
                        for c in range(CS2):
                            nc.tensor.matmul(
                                aggp_ps[:],
                                lhsT=sel[:, sub * CS2 + c, :],
                                rhs=gat[:, sub * CS2 + c, :],
                                start=(c == 0), stop=(c == CS2 - 1))
                        aggp_sb = sp.tile([P, OUT_DIM], dt.bfloat16,
                                          name=f"aggsb{h}", tag=f"aggsb{h}",
                                          bufs=(2 * LAG + 4) if h == 0 else 3)
                        nc.scalar.activation(aggp_sb[:], aggp_ps[:], AF.Copy)
                        aggs.append(aggp_sb)
                    return aggs

                g0_stash = {}
                a0_stash = {}
                for it in range(NST + LAG):
                    if it < NST:
                        g0 = issue_gathers(it, 0)
                        g0_stash[it] = g0
                        a0_stash[it] = agg_half(it, 0, g0)
                        g0_stash.pop(it)
                    if it < LAG:
                        continue
                    st = it - LAG
                    r0 = st * ST
                    g1 = issue_gathers(st, 1)
                    aggs1 = agg_half(st, 1, g1)
                    aggs0 = a0_stash.pop(st)

                    for sub in range(2):
                        rr = r0 + sub * P
                        outT_ps = pp.tile([P, P], dt.float32, name="outT_ps",
                                          tag="outT_ps", bufs=2)
                        nc.tensor.matmul(outT_ps[:], lhsT=w2_sb[:, 0, :],
                                         rhs=hta[:, rr:rr + P], start=True, stop=False)
                        nc.tensor.matmul(outT_ps[:], lhsT=w2_sb[:, 1, :],
                                         rhs=htb[:, rr:rr + P], start=False, stop=False)
                        nc.tensor.matmul(outT_ps[:], lhsT=aggs0[sub][:],
                                         rhs=diag2_sb[:, rr:rr + P],
                                         start=False, stop=False)
                        nc.tensor.matmul(outT_ps[:], lhsT=aggs1[sub][:],
                                         rhs=diag2_sb[:, rr:rr + P],
                                         start=False, stop=True)
                        o_sb = sp.tile([P, P], dt.bfloat16, name="o_sb", tag="o_sb", bufs=3)
                        nc.scalar.activation(o_sb[:], outT_ps[:], AF.Relu,
                                             bias=b2_sb[:, 0:1])
                        nc.sync.dma_start(t_out.ap()[:, rr:rr + P], o_sb[:])

    nc.compile()
    _PROGRAM_CACHE[key] = nc
    return nc


# ----------------------------------------------------------------------------
# entry point
# ----------------------------------------------------------------------------

def kernel(x, W1, b1, W2, b2, edge_src0, edge_dst0, edge_src1, edge_dst1,
           _want_results=False, **_ignored):
    CS1, CS2, in_maps = _preprocess(x, W1, b1, W2, b2,
                                    edge_src0, edge_dst0, edge_src1, edge_dst1)
    nc = build_program(CS1, CS2)
    res = run_bass_kernel_spmd(nc, in_maps, core_ids=list(range(NCORES)))
    out = np.concatenate(
        [res.results[c]["out"][:, :NPC].T for c in range(NCORES)], axis=0)
    out = np.ascontiguousarray(out, dtype=np.float32)
    if _want_results:
        return out, res
    return out


# revision 20
# speedup vs baseline: 1.0066x; 1.0066x over previous
"""GraphSAGE 2-layer encoder on 8 Trainium2 NeuronCores (Bass/Tile).

Strategy (graph/data parallel, dst-sharded):
  - Nodes are range-sharded across 8 cores (6250 each, padded to 6400).
  - Mean aggregation runs on the TensorEngine via one-hot selection
    matrices: for each chunk of 128 edges, sel[e, n] = (dst_local[e]==n).
    All sel chunks of a (supertile, 128-subtile) are built by ONE wide
    DVE tensor_tensor is_equal with broadcast access patterns
    (in0 = iota row tile broadcast across chunks, in1 = per-chunk dst
    columns broadcast across the 128 one-hot lanes) — no per-chunk
    tensor_scalar, no fp32 scalar tables.
  - The 1/deg mean normalization is NOT baked into sel. Layer 1 folds it
    into the PSUM->SBUF move (one DVE tensor_tensor mult against a
    replicated-invdeg table). Layer 2 accumulates agg row-major
    (lhsT=sel) and folds normalization + accumulation into the output
    PSUM group with a single matmul against a diagonal-invdeg tile.
  - Per-edge source features are fetched with gpsimd dma_gather (int16
    indices, so gather tables are split in two <32768-row halves).
  - Layer 2 uses the projection identity agg2 @ W2b = D^-1 A2 (h @ W2b):
    each core projects its own h rows to p = h @ W2b (128-dim) on the
    PE (row-major, directly from the resident transposed h), AllGathers
    p (half the bytes of h), and gathers 256-byte p rows per edge.
    out^T = relu(W2a^T h^T + D^-1 aggp^T + b2) is produced transposed
    [128, NPAD]; the host transposes back.
  - The p AllGather is split in two chunks (separate Shared tensors,
    which double as the int16 lo/hi gather-table halves); chunk 0 is
    issued as soon as supertile 12 of layer 1 completes so it overlaps
    the tail of layer 1.

The Bass program is identical for all cores and independent of the edge
data (given fixed chunk capacities) — per-core behavior comes only from
the input tables.
"""

import numpy as np
import ml_dtypes

import concourse.bass as bass
import concourse.mybir as mybir
import concourse.tile as tile
from concourse import bacc
from concourse.bass import AP
from concourse.bass_utils import run_bass_kernel_spmd
from concourse.masks import make_identity

BF16 = ml_dtypes.bfloat16

# problem constants (hardcoded per contract)
N = 50000
E = 800000
IN_DIM = 128
HID = 256
OUT_DIM = 128

NCORES = 8
NPC = N // NCORES          # 6250 nodes per core
ST = 256                   # supertile (dst nodes per outer loop iteration)
NST = 25                   # supertiles per core (6400 padded rows)
NPAD = NST * ST            # 6400
HALF_X = N // 2            # 25000: layer-1 gather-table half size
BLK0 = NPC // 2            # 3125: allgather chunk-0 rows per core
BLK1 = NPC - BLK0          # 3125: chunk-1 rows (exposed tail collective)
P = 128

CSUB_DEFAULT = 9           # chunks of 128 edges per (128-subtile, half)
GSPLIT = 6                 # chunks per dma_gather instruction
AG0_ST = 12                # first supertile covering all of chunk-0 rows
LAG = 2                    # layer-2 half-1 gather delay (supertiles)

_PROGRAM_CACHE: dict = {}


def _bcast_free(ap, reps, axis_pos):
    """Insert a [0, reps] broadcast free dim at axis_pos into an AP."""
    new = AP(ap.tensor, ap.offset, [list(p) for p in ap.ap])
    new.ap.insert(axis_pos, [0, reps])
    return new


# ----------------------------------------------------------------------------
# host-side preprocessing
# ----------------------------------------------------------------------------

def _pack_layer(src, dst, half_of, rel_of, csub_default):
    """Build per-core gather/selection tables for one layer.

    Edges are grouped by (core, 256-supertile, table-half, 128-subtile),
    each group padded to CSUB chunks of 128 edges (pad edges gather row 0,
    dst slot 0 of a zero-weight... pad slots point at one-hot column of
    dst 0 but their gathered row is multiplied into column 0 only via
    sel; normalization tables make pad contributions vanish because pad
    slots use dst value -1 (no one-hot match)).

    Column orders differ per consumer:
      gather idx cols:  (st, half, sub, c)  [h-pure runs for dma_gather]
      sel dst cols:     (st, sub, half, c)  [contiguous run per wide TT]

    Returns (CSUB, idx_tabs, dst_tabs); per core:
      idx_tab [128, NST*2*(2*CSUB)*8] int16 (16-partition wrap, tiled x8)
      dst_tab [128, NST*4*CSUB] bf16 (dst one-hot targets, -1 for pads)
    """
    core = dst // NPC
    loc = dst - core * NPC                   # 0..6249
    st = loc >> 8                            # supertile 0..24
    sub = (loc >> 7) & 1                     # 128-subtile within supertile
    dst_in = (loc & 127).astype(np.float32)
    half = half_of[src].astype(np.int64)
    rel = rel_of[src].astype(np.int16)

    group = ((core * NST + st) * 2 + half) * 2 + sub
    ngroups = NCORES * NST * 4
    counts = np.bincount(group, minlength=ngroups)
    CSUB = max(csub_default, int(-(-counts.max() // P)))
    cap = CSUB * P

    order = np.argsort(group, kind="stable")
    g_sorted = group[order]
    starts = np.concatenate([[0], np.cumsum(counts)])
    slot = np.arange(E) - starts[g_sorted]

    idx_pad = np.zeros((ngroups, cap), dtype=np.int16)
    dst_pad = np.full((ngroups, cap), -1.0, dtype=np.float32)
    idx_pad[g_sorted, slot] = rel[order]
    dst_pad[g_sorted, slot] = dst_in[order]

    # group index decomposition: g = ((cs * NST + st) * 2 + h) * 2 + sub
    gs = np.arange(ngroups)
    g_core, g_rem = gs // (NST * 4), gs % (NST * 4)
    g_st, g_hs = g_rem // 4, g_rem % 4
    g_h, g_sub = g_hs // 2, g_hs % 2

    # idx wrap: linear i -> (partition i%16, col i//16); [16, CSUB*8] per group
    idx_w = idx_pad.reshape(ngroups, CSUB * 8, 16).transpose(0, 2, 1)
    # dst chunk cols: linear i -> (partition i%128, chunk i//128)
    dst_w = dst_pad.reshape(ngroups, CSUB, P).transpose(0, 2, 1)   # [g,128,CSUB]

    # gather order (st, h, sub, c) / sel order (st, sub, h, c)
    gath_order = np.lexsort((g_sub, g_h, g_st, g_core))
    sel_order = np.lexsort((g_h, g_sub, g_st, g_core))

    idx_tabs, dst_tabs = [], []
    npg = NST * 4
    for c in range(NCORES):
        go = gath_order[c * npg:(c + 1) * npg]
        so = sel_order[c * npg:(c + 1) * npg]
        it = idx_w[go].transpose(1, 0, 2).reshape(16, -1)
        idx_tabs.append(np.ascontiguousarray(np.tile(it, (8, 1))))
        dst_tabs.append(np.ascontiguousarray(
            dst_w[so].transpose(1, 0, 2).reshape(P, -1)).astype(BF16))
    return CSUB, idx_tabs, dst_tabs


def _invdeg_tables(dst_edges):
    """Per-core [128, NPAD] bf16: replicated-row and diagonal invdeg."""
    deg = np.bincount(dst_edges, minlength=N)
    invdeg = (1.0 / np.maximum(deg, 1.0)).astype(np.float32)
    rep_tabs, diag_tabs = [], []
    for c in range(NCORES):
        v = np.zeros(NPAD, np.float32)
        v[:NPC] = invdeg[c * NPC:(c + 1) * NPC]
        rep = np.broadcast_to(v, (P, NPAD))
        dia = np.zeros((P, NPAD), np.float32)
        blocks = v.reshape(NPAD // P, P)
        for b in range(NPAD // P):
            dia[:, b * P:(b + 1) * P] = np.diag(blocks[b])
        rep_tabs.append(np.ascontiguousarray(rep).astype(BF16))
        diag_tabs.append(np.ascontiguousarray(dia).astype(BF16))
    return rep_tabs, diag_tabs


def _preprocess(x, W1, b1, W2, b2, es0, ed0, es1, ed1):
    x = np.asarray(x, dtype=np.float32)
    es0 = np.asarray(es0, dtype=np.int64)
    ed0 = np.asarray(ed0, dtype=np.int64)
    es1 = np.asarray(es1, dtype=np.int64)
    ed1 = np.asarray(ed1, dtype=np.int64)

    # layer 1: table is x itself, node n at row n; halves split at 25000
    ident_rows = np.arange(N, dtype=np.int64)
    half1 = (ident_rows >= HALF_X).astype(np.int64)
    rel1 = ident_rows - half1 * HALF_X
    CS1, idx1, dst1 = _pack_layer(es0, ed0, half1, rel1, CSUB_DEFAULT)
    rep1, _ = _invdeg_tables(ed0)

    # layer 2: two p_full chunks: node n = c*NPC + j lives in chunk 0 at
    # row c*BLK0 + j (j < BLK0), else chunk 1 at c*BLK1 + (j - BLK0)
    c_of = ident_rows // NPC
    j_of = ident_rows % NPC
    half2 = (j_of >= BLK0).astype(np.int64)
    rel2 = np.where(half2 == 0, c_of * BLK0 + j_of,
                    c_of * BLK1 + (j_of - BLK0))
    CS2, idx2, dst2 = _pack_layer(es1, ed1, half2, rel2, CSUB_DEFAULT)
    _, diag2 = _invdeg_tables(ed1)

    x_bf = x.astype(BF16)
    xownt = []
    for c in range(NCORES):
        xo = np.zeros((NPAD, IN_DIM), dtype=np.float32)
        xo[:NPC] = x[c * NPC:(c + 1) * NPC]
        xownt.append(np.ascontiguousarray(xo.T).astype(BF16))

    W1_bf = np.asarray(W1, np.float32).astype(BF16)           # [256, 256]
    W2_bf = np.asarray(W2, np.float32).astype(BF16)           # [512, 128]
    b1_2 = np.asarray(b1, np.float32).reshape(2, P).T.copy()  # [128, 2]
    b2_c = np.asarray(b2, np.float32).reshape(P, 1).copy()    # [128, 1]

    in_maps = []
    for c in range(NCORES):
        in_maps.append({
            "xtab": x_bf,
            "xownt": xownt[c],
            "w1": W1_bf,
            "w2": W2_bf,
            "b1": b1_2,
            "b2": b2_c,
            "idx1": idx1[c], "dst1": dst1[c], "rep1": rep1[c],
            "idx2": idx2[c], "dst2": dst2[c], "diag2": diag2[c],
        })
    return CS1, CS2, in_maps


# ----------------------------------------------------------------------------
# device program
# ----------------------------------------------------------------------------

def build_program(CS1, CS2, ablate=()):
    key = (CS1, CS2, tuple(sorted(ablate)))
    if key in _PROGRAM_CACHE:
        return _PROGRAM_CACHE[key]

    C1, C2 = 2 * CS1, 2 * CS2          # chunks per (supertile, half)
    S1, S2 = 2 * C1, 2 * C2            # chunk slots per supertile
    dt = mybir.dt
    AF = mybir.ActivationFunctionType
    nc = bacc.Bacc("TRN2", target_bir_lowering=False, debug=False,
                   num_devices=NCORES, num_swdge_queues=4,
                   dynamic_dma_scratch_size=32768)

    t_xtab = nc.dram_tensor("xtab", [N, IN_DIM], dt.bfloat16, kind="ExternalInput")
    t_xownt = nc.dram_tensor("xownt", [IN_DIM, NPAD], dt.bfloat16, kind="ExternalInput")
    t_w1 = nc.dram_tensor("w1", [HID, HID], dt.bfloat16, kind="ExternalInput")
    t_w2 = nc.dram_tensor("w2", [2 * HID, OUT_DIM], dt.bfloat16, kind="ExternalInput")
    t_b1 = nc.dram_tensor("b1", [P, 2], dt.float32, kind="ExternalInput")
    t_b2 = nc.dram_tensor("b2", [P, 1], dt.float32, kind="ExternalInput")
    t_idx1 = nc.dram_tensor("idx1", [P, NST * 2 * C1 * 8], dt.int16, kind="ExternalInput")
    t_dst1 = nc.dram_tensor("dst1", [P, NST * S1], dt.bfloat16, kind="ExternalInput")
    t_rep1 = nc.dram_tensor("rep1", [P, NPAD], dt.bfloat16, kind="ExternalInput")
    t_idx2 = nc.dram_tensor("idx2", [P, NST * 2 * C2 * 8], dt.int16, kind="ExternalInput")
    t_dst2 = nc.dram_tensor("dst2", [P, NST * S2], dt.bfloat16, kind="ExternalInput")
    t_diag2 = nc.dram_tensor("diag2", [P, NPAD], dt.bfloat16, kind="ExternalInput")
    t_out = nc.dram_tensor("out", [OUT_DIM, NPAD], dt.bfloat16, kind="ExternalOutput")

    qctr = [0]
    with tile.TileContext(nc) as tc:
        with tc.tile_pool(name="const", bufs=1) as cp, \
             tc.tile_pool(name="dram", bufs=1, space="DRAM") as dp:

            # ---- constants / persistent SBUF ----
            ident_bf = cp.tile([P, P], dt.bfloat16, name="ident_bf")
            make_identity(nc, ident_bf)
            iota_i = cp.tile([P, P], dt.int32, name="iota_i")
            nc.gpsimd.iota(iota_i, pattern=[[1, P]], base=0, channel_multiplier=0)
            iota_bf = cp.tile([P, P], dt.bfloat16, name="iota_bf")
            nc.vector.tensor_copy(iota_bf[:], iota_i[:])

            # layer-1 gather indices load first (and in two pieces) so the
            # first gathers issue within a few microseconds of program start
            idx1_sb = cp.tile([P, NST * 2 * C1 * 8], dt.int16, name="idx1_sb")
            c_split = 4 * C1 * 8
            nc.sync.dma_start(idx1_sb[:, 0:c_split], t_idx1.ap()[:, 0:c_split])
            nc.sync.dma_start(idx1_sb[:, c_split:], t_idx1.ap()[:, c_split:])

            # w2 rows: a=0,1 -> W2a halves; a=2,3 -> W2b halves
            w1_sb = cp.tile([P, 2, HID], dt.bfloat16, name="w1_sb")
            nc.sync.dma_start(w1_sb[:], t_w1.ap().rearrange("(a p) h -> p a h", p=P))
            w2_sb = cp.tile([P, 4, OUT_DIM], dt.bfloat16, name="w2_sb")
            nc.sync.dma_start(w2_sb[:], t_w2.ap().rearrange("(a p) h -> p a h", p=P))
            b1_sb = cp.tile([P, 2], dt.float32, name="b1_sb")
            nc.sync.dma_start(b1_sb[:], t_b1.ap()[:])
            b2_sb = cp.tile([P, 1], dt.float32, name="b2_sb")
            nc.sync.dma_start(b2_sb[:], t_b2.ap()[:])

            xt_all = cp.tile([P, NPAD], dt.bfloat16, name="xt_all")
            nc.sync.dma_start(xt_all[:], t_xownt.ap()[:])
            dst1_sb = cp.tile([P, NST * S1], dt.bfloat16, name="dst1_sb")
            nc.sync.dma_start(dst1_sb[:], t_dst1.ap()[:])
            rep1_sb = cp.tile([P, NPAD], dt.bfloat16, name="rep1_sb")
            nc.sync.dma_start(rep1_sb[:], t_rep1.ap()[:])
            idx2_sb = cp.tile([P, NST * 2 * C2 * 8], dt.int16, name="idx2_sb")
            nc.sync.dma_start(idx2_sb[:], t_idx2.ap()[:])
            dst2_sb = cp.tile([P, NST * S2], dt.bfloat16, name="dst2_sb")
            nc.sync.dma_start(dst2_sb[:], t_dst2.ap()[:])
            diag2_sb = cp.tile([P, NPAD], dt.bfloat16, name="diag2_sb")
            nc.sync.dma_start(diag2_sb[:], t_diag2.ap()[:])

            # persistent transposed h (self-features for layer 2)
            hta = cp.tile([P, NPAD], dt.bfloat16, name="hta")
            htb = cp.tile([P, NPAD], dt.bfloat16, name="htb")

            # allgather chunk shards + gathered tables
            blks = (BLK0, BLK1)
            pshs = [
                dp.tile([blks[k], OUT_DIM], dt.bfloat16, name=f"psh{k}")
                for k in range(2)
            ]
            pfulls = [
                dp.tile([NCORES * blks[k], OUT_DIM], dt.bfloat16,
                        name=f"pfull{k}", addr_space="Shared")
                for k in range(2)
            ]

            def do_allgather(k):
                nc.gpsimd.collective_compute(
                    "AllGather",
                    mybir.AluOpType.bypass,
                    replica_groups=[list(range(NCORES))],
                    ins=[pshs[k][:].opt()],
                    outs=[pfulls[k][:].opt()],
                )

            def build_sel_wide(sp, dst_sb, st, sub, CS, tag, half=None, bufs=3):
                """One wide TT is_equal building one-hot sel chunks.

                half=None: all 2*CS chunks of (st, sub), dst cols ordered
                (st, sub, h, c). half=h: the CS chunks of (st, sub=0/1, h)
                for BOTH subs -> [128, 2*CS, 128] with chunk dim (sub, c),
                using a 4-dim strided in1 (cols (st, sub, h, c))."""
                sel = sp.tile([P, 2 * CS, P], dt.bfloat16, name=tag, tag=tag, bufs=bufs)
                if half is None:
                    base = (st * 2 + sub) * (2 * CS)
                    in0 = _bcast_free(iota_bf[:], 2 * CS, 1)
                    in1 = _bcast_free(dst_sb[:, base:base + 2 * CS], P, 2)
                else:
                    base = (st * 4 + half) * CS
                    in0 = _bcast_free(_bcast_free(iota_bf[:], CS, 1), 2, 1)
                    cols = dst_sb[:, base:base + CS]
                    # dims: [part, (sub: stride 2*CS cols), (c: 1), (bcast 128)]
                    cstride = cols.ap[-1][0]
                    in1 = AP(cols.tensor, cols.offset,
                             [list(cols.ap[0]), [2 * CS * cstride, 2],
                              [cstride, CS], [0, P]])
                nc.vector.tensor_tensor(out=sel[:], in0=in0, in1=in1,
                                        op=mybir.AluOpType.is_equal)
                return sel

            # ---- layer 1 ----
            with tc.tile_pool(name="l1sb", bufs=2) as sp, \
                 tc.tile_pool(name="l1ps", bufs=2, space="PSUM") as pp:
                for st in range(NST):
                    r0 = st * ST

                    # gathers: chunk dim of gat is (sub, c) within half h
                    gats = []
                    for h in range(2):
                        g = st * 2 + h
                        gat = sp.tile([P, C1, IN_DIM], dt.bfloat16,
                                      name=f"gat{h}", tag=f"gat{h}", bufs=4)
                        if "gather" in ablate:
                            nc.vector.memset(gat[:], 0.0)
                        for j in range(0, C1, GSPLIT):
                            if "gather" in ablate:
                                continue
                            w = min(GSPLIT, C1 - j)
                            nc.gpsimd.dma_gather(
                                out_ap=gat[:, j:j + w, :],
                                in_ap=t_xtab.ap()[h * HALF_X:(h + 1) * HALF_X, :],
                                idxs_ap=idx1_sb[:, (g * C1 + j) * 8:(g * C1 + j + w) * 8],
                                num_idxs=w * P,
                                num_idxs_reg=w * P,
                                elem_size=IN_DIM,
                                queue_num=qctr[0] % 4,
                            )
                            qctr[0] += 1
                        gats.append(gat)

                    # one [128,128] psum tile per 128-subtile, A then B
                    aggT = sp.tile([P, ST], dt.bfloat16, name="aggT", tag="aggT")
                    for sub in range(2):
                        sel = build_sel_wide(sp, dst1_sb, st, sub, CS1, "sel")
                        aggT_ps = pp.tile([P, P], dt.float32, name="aggT_ps", tag="aggT_ps", bufs=3)
                        n_mm = 2 * CS1
                        k = 0
                        for h in range(2):
                            for c in range(CS1):
                                nc.tensor.matmul(
                                    aggT_ps[:],
                                    lhsT=gats[h][:, sub * CS1 + c, :],
                                    rhs=sel[:, h * CS1 + c, :],
                                    start=(k == 0), stop=(k == n_mm - 1))
                                k += 1
                        # fused PSUM->SBUF copy + per-node invdeg scale
                        nc.vector.tensor_tensor(
                            out=aggT[:, sub * P:(sub + 1) * P], in0=aggT_ps[:],
                            in1=rep1_sb[:, r0 + sub * P:r0 + (sub + 1) * P],
                            op=mybir.AluOpType.mult)

                    # hT = relu(W1^T @ [x; agg] + b1), two hid halves
                    for hh, hstore in ((0, hta), (1, htb)):
                        hT_ps = pp.tile([P, ST], dt.float32, name="hT_ps", tag="hT_ps")
                        nc.tensor.matmul(hT_ps[:], lhsT=w1_sb[:, 0, hh * P:(hh + 1) * P],
                                         rhs=xt_all[:, r0:r0 + ST], start=True, stop=False)
                        nc.tensor.matmul(hT_ps[:], lhsT=w1_sb[:, 1, hh * P:(hh + 1) * P],
                                         rhs=aggT[:], start=False, stop=True)
                        nc.scalar.activation(hstore[:, r0:r0 + ST], hT_ps[:],
                                             AF.Relu, bias=b1_sb[:, hh:hh + 1])

                    # p rows = h @ W2b (row-major via lhsT = resident hT)
                    for nh in range(2):
                        rr = r0 + nh * P
                        if rr >= NPC:
                            continue
                        p_ps = pp.tile([P, OUT_DIM], dt.float32, name="p_ps", tag="p_ps", bufs=3)
                        nc.tensor.matmul(p_ps[:], lhsT=hta[:, rr:rr + P],
                                         rhs=w2_sb[:, 2, :], start=True, stop=False)
                        nc.tensor.matmul(p_ps[:], lhsT=htb[:, rr:rr + P],
                                         rhs=w2_sb[:, 3, :], start=False, stop=True)
                        p_sb = sp.tile([P, OUT_DIM], dt.bfloat16, name="p_sb", tag="p_sb", bufs=3)
                        nc.scalar.activation(p_sb[:], p_ps[:], AF.Copy)
                        # split the store at the allgather chunk boundary
                        for seg_lo, seg_hi, t_dst in ((0, BLK0, pshs[0]),
                                                      (BLK0, NPC, pshs[1])):
                            lo = max(rr, seg_lo)
                            hi = min(rr + P, seg_hi)
                            if hi > lo:
                                nc.sync.dma_start(
                                    t_dst[lo - seg_lo:hi - seg_lo, :],
                                    p_sb[lo - rr:hi - rr, :])

                    if st == AG0_ST:
                        do_allgather(0)

            do_allgather(1)

            # ---- layer 2 ----
            # aggp accumulates row-major [node, out] (lhsT=sel chunks); the
            # output PSUM group stacks the dense self terms then one matmul
            # against the diagonal-invdeg tile which both normalizes and
            # accumulates the aggregation term.
            with tc.tile_pool(name="l2sb", bufs=2) as sp, \
                 tc.tile_pool(name="l2ps", bufs=2, space="PSUM") as pp:
                def issue_gathers(st, h):
                    g = st * 2 + h
                    gat = sp.tile([P, C2, OUT_DIM], dt.bfloat16,
                                  name=f"g2_{h}", tag=f"g2_{h}",
                                  bufs=4 if h == 0 else 3)
                    if "gather" in ablate:
                        nc.vector.memset(gat[:], 0.0)
                        return gat
                    for j in range(0, C2, GSPLIT):
                        w = min(GSPLIT, C2 - j)
                        nc.gpsimd.dma_gather(
                            out_ap=gat[:, j:j + w, :],
                            in_ap=pfulls[h][:],
                            idxs_ap=idx2_sb[:, (g * C2 + j) * 8:(g * C2 + j + w) * 8],
                            num_idxs=w * P,
                            num_idxs_reg=w * P,
                            elem_size=OUT_DIM,
                            queue_num=qctr[0] % 4,
                        )
                        qctr[0] += 1
                    return gat

                # Half-0 gathers run LAG supertiles ahead so the Pool engine
                # keeps gathering from pfull0 while the chunk-1 allgather is
                # still in flight (in-order queue would otherwise block on the
                # first pfull1 gather). The aggregation PSUM is split per half
                # so half-0 matmuls consume (and release) their gather tiles
                # without waiting on half-1.
                def agg_half(st, h, gat):
                    sel = build_sel_wide(sp, dst2_sb, st, 2, CS2, f"sel2_{h}",
                                         half=h, bufs=2)
                    aggs = []
                    for sub in range(2):
                        aggp_ps = pp.tile([P, OUT_DIM], dt.float32,
                                          name=f"aggp{h}", tag=f"aggp{h}", bufs=2)
                        for c in range(CS2):
                            nc.tensor.matmul(
                                aggp_ps[:],
                                lhsT=sel[:, sub * CS2 + c, :],
                                rhs=gat[:, sub * CS2 + c, :],
                                start=(c == 0), stop=(c == CS2 - 1))
                        aggp_sb = sp.tile([P, OUT_DIM], dt.bfloat16,
                                          name=f"aggsb{h}", tag=f"aggsb{h}",
                                          bufs=(2 * LAG + 4) if h == 0 else 3)
                        nc.scalar.activation(aggp_sb[:], aggp_ps[:], AF.Copy)
                        aggs.append(aggp_sb)
                    return aggs

                g0_stash = {}
                a0_stash = {}
                for it in range(NST + LAG):
                    if it < NST:
                        g0 = issue_gathers(it, 0)
                        g0_stash[it] = g0
                        a0_stash[it] = agg_half(it, 0, g0)
                        g0_stash.pop(it)
                    if it < LAG:
                        continue
                    st = it - LAG
                    r0 = st * ST
                    g1 = issue_gathers(st, 1)
                    aggs1 = agg_half(st, 1, g1)
                    aggs0 = a0_stash.pop(st)

                    for sub in range(2):
                        rr = r0 + sub * P
                        outT_ps = pp.tile([P, P], dt.float32, name="outT_ps",
                                          tag="outT_ps", bufs=2)
                        nc.tensor.matmul(outT_ps[:], lhsT=w2_sb[:, 0, :],
                                         rhs=hta[:, rr:rr + P], start=True, stop=False)
                        nc.tensor.matmul(outT_ps[:], lhsT=w2_sb[:, 1, :],
                                         rhs=htb[:, rr:rr + P], start=False, stop=False)
                        nc.tensor.matmul(outT_ps[:], lhsT=aggs0[sub][:],
                                         rhs=diag2_sb[:, rr:rr + P],
                                         start=False, stop=False)
                        nc.tensor.matmul(outT_ps[:], lhsT=aggs1[sub][:],
                                         rhs=diag2_sb[:, rr:rr + P],
                                         start=False, stop=True)
                        o_sb = sp.tile([P, P], dt.bfloat16, name="o_sb", tag="o_sb", bufs=3)
                        nc.scalar.activation(o_sb[:], outT_ps[:], AF.Relu,
                                             bias=b2_sb[:, 0:1])
                        nc.sync.dma_start(t_out.ap()[:, rr:rr + P], o_sb[:])

    nc.compile()
    _PROGRAM_CACHE[key] = nc
    return nc


# ----------------------------------------------------------------------------
# entry point
# ----------------------------------------------------------------------------

def kernel(x, W1, b1, W2, b2, edge_src0, edge_dst0, edge_src1, edge_dst1,
           _want_results=False, **_ignored):
    CS1, CS2, in_maps = _preprocess(x, W1, b1, W2, b2,
                                    edge_src0, edge_dst0, edge_src1, edge_dst1)
    nc = build_program(CS1, CS2)
    res = run_bass_kernel_spmd(nc, in_maps, core_ids=list(range(NCORES)))
    out = np.concatenate(
        [res.results[c]["out"][:, :NPC].T for c in range(NCORES)], axis=0)
    out = np.ascontiguousarray(out, dtype=np.float32)
    if _want_results:
        return out, res
    return out
